# revision 2
# baseline (speedup 1.0000x reference)
"""LocalExpansion (7x7 unfold) Trainium2 Bass kernel.

Full input x: [2, 8, 2304, 64] f32 (B=2, heads=8, N=48*48, D=64).
Full output:  [2, 8, 2304, 49, 64] f32 — out[b,h,y*W+x,i*7+j,:] =
x_img[b,h,y+i-3,x+j-3,:] with zero fill outside the 48x48 image.

Strategy (pure DMA, memory-regime):
- batch*heads = 16 images, 2 per core across 8 NeuronCores.
- Per core: zero-pad each 48x48x64 image into SBUF as 54 rows
  (one padded row per partition, 54*64 floats = 13824 B). Image 0 on
  partitions 0-53 (even-SDMA-engine half), image 1 on partitions
  64-117 (odd half) so concurrent DMAs load all 16 SDMA engines.
- For each filter row i (7 of them) one 3D DMA writes the whole
  [48 y, 48 x, 7*64 floats] slab: src is an overlapping sliding
  window (x stride 64 floats < 448-float element) read from SBUF,
  dst is strided DRAM with 1792 B contiguous chunks. Boundary zeros
  come for free from the padded SBUF image.
HBM traffic per core = 57.8 MB writes + 1.2 MB reads (~roofline).
"""

import numpy as np

KH, KW = 7, 7
H, W, D = 48, 48, 64
PH, PW = H + 6, W + 6          # 54x54 padded image
ROW = PW * D                   # floats per padded row (one SBUF partition)
N = H * W                      # 2304
K = KH * KW                    # 49
IMG_OUT = N * K * D            # floats per image output
IMGS_PER_CORE = 2
N_CORES = 8
BASES = (0, 64)                # SBUF base partitions per image

_CACHE = {}


def _build_nc():
    import concourse.bass as bass
    import concourse.mybir as mybir

    nc = bass.Bass(trn_type="TRN2")
    x = nc.dram_tensor("x", [IMGS_PER_CORE, N, D], mybir.dt.float32,
                       kind="ExternalInput")
    out = nc.dram_tensor("out", [IMGS_PER_CORE, N, K, D], mybir.dt.float32,
                         kind="ExternalOutput")

    with (
        nc.sbuf_tensor("pad", [128, ROW], mybir.dt.float32) as pad,
        nc.semaphore("ld") as ld,
        nc.semaphore("ms") as ms,
        nc.semaphore("st") as st,
    ):
        # Zero the whole padded buffer once (pad strips stay zero), then
        # load both images into the padded interiors.
        nc.vector.memset(
            bass.AP(pad, 0, [[ROW, 128], [1, ROW]]), 0.0
        ).then_inc(ms, 1)
        nc.sync.wait_ge(ms, 1)
        for im in range(IMGS_PER_CORE):
            bp = BASES[im]
            nc.sync.dma_start(
                out=bass.AP(pad, (bp + 3) * ROW + 3 * D, [[ROW, H], [1, W * D]]),
                in_=bass.AP(x, im * N * D, [[W * D, H], [1, W * D]]),
            ).then_inc(ld, 16)

        nc.sync.wait_ge(ld, IMGS_PER_CORE * 16)

        # 7 filter-row slabs per image; interleave images so both SDMA
        # engine halves (even: partitions 0-63, odd: 64-127) stay busy.
        n_st = 0
        for i in range(KH):
            for im in range(IMGS_PER_CORE):
                bp = BASES[im]
                nc.sync.dma_start(
                    out=bass.AP(
                        out,
                        im * IMG_OUT + i * KW * D,
                        [[W * K * D, H], [K * D, W], [1, KW * D]],
                    ),
                    in_=bass.AP(
                        pad,
                        (bp + i) * ROW,
                        [[ROW, H], [D, W], [1, KW * D]],
                    ),
                ).then_inc(st, 16)
                n_st += 16
        nc.sync.wait_ge(st, n_st)
    return nc


def kernel(x, height=48, width=48):
    from concourse.bass_utils import run_bass_kernel_spmd

    x = np.asarray(x)
    b, nh = x.shape[0], x.shape[1]
    xi = np.ascontiguousarray(x.reshape(b * nh, N, D))
    in_maps = [
        {"x": np.ascontiguousarray(xi[IMGS_PER_CORE * c: IMGS_PER_CORE * (c + 1)])}
        for c in range(N_CORES)
    ]
    if "nc" not in _CACHE:
        _CACHE["nc"] = _build_nc()
    res = run_bass_kernel_spmd(_CACHE["nc"], in_maps, core_ids=list(range(N_CORES)))
    y = np.stack([res.results[c]["out"] for c in range(N_CORES)])
    return y.reshape(b, nh, N, K, D).astype(np.float32, copy=False)
